# revision 6
# baseline (speedup 1.0000x reference)
"""GQA (32 Q heads / 8 KV heads, S=2048, D=4096, hd=128) on 8 TRN2 cores.

Tensor-parallel over heads: core c owns Q heads [4c, 4c+4) and KV head c.
v3 changes over baseline:
  - phase 1 uses 512-wide seq panels (half the PE instruction count; N=512
    matmuls are the engine-bound regime measured on HW)
  - phase 2 rowsum matmuls use 4-way column tiling (tile_position col
    groups, psum out partitions 0/32/64/96) so 4 adjacent rowsum matmuls
    run concurrently in the PE array; the rowsum psum bank is cleared by
    one full-width zero-stationary start=True matmul per (q-chunk, head)
    -- the only clearing scheme with identical semantics on HW (bank-wide
    has_written clear) and CoreSim (uniform pending-zero); a DVE memset
    to PSUM is a silent no-op on HW
  - phase 3 (output projection) is interleaved into the phase-2 software
    pipeline as exp-independent PE work; evacuation alternates ACT/DVE
  - woT loads at the start of phase 2 (SBUF too tight during phase 1)
Host sums the 8 partial outputs in fp32 and transposes back.
"""

import numpy as np

import concourse.bass as bass
import concourse.mybir as mybir
import concourse.tile as tile
from concourse import bacc
from concourse.bass_utils import run_bass_kernel_spmd
from concourse.masks import make_identity

B, S, D = 1, 2048, 4096
N_HEADS, N_KV = 32, 8
HD = 128                      # head dim
GROUP = N_HEADS // N_KV       # 4
NCORES = 8
HPC = N_HEADS // NCORES       # 4 q heads per core
QO = HPC * HD                 # 512 q rows per core
SCALE = 1.0 / np.sqrt(np.float32(HD))

SP = 512                      # phase-1 seq panel width
NSP = S // SP                 # 4 panels
DCH = D // 128                # 32 contraction chunks
IC = 512                      # phase-2 query chunk width
NIC = S // IC                 # 4 query chunks
NJB = S // 128                # 16 key blocks
NOB = D // 128                # 32 output blocks

BF = mybir.dt.bfloat16
F32 = mybir.dt.float32

RS_GROUPS = 4  # rowsum col-tile groups (partitions 0/32/64/96)
ROWSUM_COLTILE = True         # 4-way col-tiled rowsum matmuls
FUSE_P3 = True                # interleave out-projection into phase 2


def _build_nc():
    nc = bacc.Bacc("TRN2", target_bir_lowering=False, debug=False)

    xT = nc.dram_tensor("xT", [D, S], BF, kind="ExternalInput")
    cosT = nc.dram_tensor("cosT", [HD, S], BF, kind="ExternalInput")
    sinTs = nc.dram_tensor("sinTs", [HD, S], BF, kind="ExternalInput")
    wqT = nc.dram_tensor("wqT", [HPC, 128, DCH, HD], BF, kind="ExternalInput")
    wkT = nc.dram_tensor("wkT", [128, DCH, HD], BF, kind="ExternalInput")
    wvT = nc.dram_tensor("wvT", [128, DCH, HD], BF, kind="ExternalInput")
    woT = nc.dram_tensor("woT", [128, HPC, D], BF, kind="ExternalInput")
    masks = nc.dram_tensor("masks", [128, 128], BF, kind="ExternalInput")
    outT = nc.dram_tensor("outT", [D, S], BF, kind="ExternalOutput")

    with tile.TileContext(nc) as tc:
        _emit(nc, tc, xT, cosT, sinTs, wqT, wkT, wvT, woT, masks, outT)
    nc.compile()
    return nc


def _emit(nc, tc, xT, cosT, sinTs, wqT, wkT, wvT, woT, masks, outT):
    from contextlib import ExitStack

    with ExitStack() as outer:
        ep = outer.enter_context  # persistent pools

        pers = ep(tc.tile_pool(name="pers", bufs=1))
        qTc = [pers.tile([128, HPC, IC], BF, name=f"qT{c}") for c in range(NIC)]
        kTc = [pers.tile([128, IC], BF, name=f"kT{c}") for c in range(NIC)]
        vnatc = [
            pers.tile([128, 4, HD], BF, name=f"vnat{c}") for c in range(NIC)
        ]
        ctxc = [
            pers.tile([128, HPC, IC], BF, name=f"ctx{c}") for c in range(NIC)
        ]
        mask_sb = pers.tile([128, 128], BF, name="mask_sb")
        ones_sb = pers.tile([128, 1], BF, name="ones_sb")
        zeros_sb = pers.tile([128, 128], BF, name="zeros_sb")
        ident = pers.tile([128, 128], BF, name="ident")
        warm_sb = pers.tile([1, 1], F32, name="warm_sb")

        nc.gpsimd.memset(ones_sb[:], 1.0)
        nc.gpsimd.memset(zeros_sb[:], 0.0)
        make_identity(nc, ident[:])

        # ---------------- phase 1: projections + RoPE ----------------
        with ExitStack() as p1:
            e = p1.enter_context
            wq_pool = e(tc.tile_pool(name="wq", bufs=1))
            wq_sbs = [
                wq_pool.tile([128, DCH, HD], BF, name=f"wq_sb{h}")
                for h in range(HPC)
            ]
            wk_pool = e(tc.tile_pool(name="wk", bufs=1))
            wk_sbs = [
                wk_pool.tile([128, DCH // 2, HD], BF, name=f"wk_sb{u}")
                for u in range(2)
            ]
            wv_sb = e(tc.tile_pool(name="wv", bufs=1)).tile(
                [128, DCH, HD], BF, name="wv_sb"
            )
            cs_pool = e(tc.tile_pool(name="cs", bufs=1))
            cosT_sb = cs_pool.tile([128, S], BF, name="cosT_sb")
            sinTs_sb = cs_pool.tile([128, S], BF, name="sinTs_sb")

            xph_pool = e(tc.tile_pool(name="xph", bufs=8))
            xpe_pool = e(tc.tile_pool(name="xpe", bufs=2))
            p1_psum = e(tc.tile_pool(name="p1ps", bufs=6, space="PSUM"))
            tr_psum = e(tc.tile_pool(name="trps", bufs=2, space="PSUM"))
            rtmp_pool = e(tc.tile_pool(name="rtmp", bufs=4))
            vt_pool = e(tc.tile_pool(name="vt", bufs=3))

            def load_xpan(sp):
                return [load_xpan_q(sp, u) for u in range(4)]

            def load_xpan_q(sp, u):
                # quarter-panel tiles so the first K matmuls of panel 0 can
                # start after only a quarter of the panel has landed
                src = xT[:, sp * SP : (sp + 1) * SP].rearrange(
                    "(c p) s -> p c s", p=128
                )
                t = xph_pool.tile([128, DCH // 4, SP], BF, name="xpanq")
                nc.sync.dma_start(
                    t[:], src[:, u * (DCH // 4) : (u + 1) * (DCH // 4), :]
                )
                return (t, u * (DCH // 4))

            def load_xpan_e(sp, u):
                # eighth-panel slice for the very first matmuls
                src = xT[:, sp * SP : (sp + 1) * SP].rearrange(
                    "(c p) s -> p c s", p=128
                )
                t = xpe_pool.tile([128, DCH // 8, SP], BF, name="xpane")
                nc.sync.dma_start(
                    t[:], src[:, u * (DCH // 8) : (u + 1) * (DCH // 8), :]
                )
                return (t, u * (DCH // 8))

            def xsel(xpan, d):
                for t, base in reversed(xpan):
                    if d >= base:
                        return t[:, d - base, :]
                raise AssertionError

            # DMA order = need order.
            nc.sync.dma_start(wk_sbs[0][:, : DCH // 4, :], wkT[:, : DCH // 4, :])
            x0_0a = load_xpan_e(0, 0)
            x0_0b = load_xpan_e(0, 1)
            nc.sync.dma_start(
                wk_sbs[0][:, DCH // 4 :, :], wkT[:, DCH // 4 : DCH // 2, :]
            )
            x0_1 = load_xpan_q(0, 1)
            nc.sync.dma_start(wk_sbs[1][:], wkT[:, DCH // 2 :, :])
            x0_2 = load_xpan_q(0, 2)
            x0_3 = load_xpan_q(0, 3)
            xpan_next = [x0_0a, x0_0b, x0_1, x0_2, x0_3]
            nc.sync.dma_start(wv_sb[:], wvT[:])
            nc.sync.dma_start(wq_sbs[0][:], wqT[0])
            nc.sync.dma_start(cosT_sb[:], cosT[:])
            nc.sync.dma_start(sinTs_sb[:], sinTs[:])
            nc.sync.dma_start(wq_sbs[1][:], wqT[1])
            xpan_1 = load_xpan(1)
            nc.sync.dma_start(wq_sbs[2][:], wqT[2])
            nc.sync.dma_start(wq_sbs[3][:], wqT[3])
            nc.sync.dma_start(mask_sb[:], masks[:])

            # Warm-up ops: absorb first-touch DMA waits (see baseline).
            nc.vector.tensor_copy(cosT_sb[:1, :1], cosT_sb[:1, :1])
            nc.vector.tensor_copy(sinTs_sb[:1, :1], sinTs_sb[:1, :1])
            nc.vector.tensor_copy(mask_sb[:1, :1], mask_sb[:1, :1])
            nc.scalar.activation(
                warm_sb[:], cosT_sb[:1, :1], mybir.ActivationFunctionType.Copy
            )
            nc.scalar.activation(
                warm_sb[:], warm_sb[:], mybir.ActivationFunctionType.Exp
            )

            for sp in range(NSP):
                s0 = sp * SP
                xpan = xpan_next
                if sp == 0:
                    xpan_next = xpan_1
                elif sp + 1 < NSP:
                    xpan_next = load_xpan(sp + 1)
                ch = sp  # SP == IC: one panel per chunk

                def k_block(xpan=xpan, s0=s0, ch=ch):
                    ps = p1_psum.tile([128, SP], F32, name="p1acc", tag="p1acc")
                    for d in range(DCH):
                        nc.tensor.matmul(
                            ps[:], wk_sbs[d // (DCH // 2)][:, d % (DCH // 2), :],
                            xsel(xpan, d),
                            start=(d == 0), stop=(d == DCH - 1),
                        )
                    _rope(nc, tc, rtmp_pool, ps, kTc[ch][:],
                          cosT_sb[:, s0 : s0 + SP], sinTs_sb[:, s0 : s0 + SP])

                def v_block(xpan=xpan, s0=s0, ch=ch):
                    ps = p1_psum.tile([128, SP], F32, name="p1acc", tag="p1acc")
                    for d in range(DCH):
                        nc.tensor.matmul(
                            ps[:], wv_sb[:, d, :], xsel(xpan, d),
                            start=(d == 0), stop=(d == DCH - 1),
                        )
                    vt = vt_pool.tile([128, SP], BF, name="vt")
                    nc.scalar.activation(
                        vt[:], ps[:], mybir.ActivationFunctionType.Copy
                    )
                    for b in range(SP // 128):
                        tp = tr_psum.tile([128, 128], BF, name="trp", tag="trp")
                        nc.tensor.transpose(
                            tp[:], vt[:, b * 128 : (b + 1) * 128], ident[:]
                        )
                        nc.scalar.activation(
                            vnatc[ch][:, b, :],
                            tp[:],
                            mybir.ActivationFunctionType.Copy,
                        )

                def q_block(h, xpan=xpan, s0=s0, ch=ch):
                    ps = p1_psum.tile([128, SP], F32, name="p1acc", tag="p1acc")
                    for d in range(DCH):
                        nc.tensor.matmul(
                            ps[:],
                            wq_sbs[h][:, d, :],
                            xsel(xpan, d),
                            start=(d == 0),
                            stop=(d == DCH - 1),
                        )
                    _rope(nc, tc, rtmp_pool, ps, qTc[ch][:, h, :],
                          cosT_sb[:, s0 : s0 + SP], sinTs_sb[:, s0 : s0 + SP])

                if sp < NSP - 1:
                    k_block()
                    v_block()
                    for h in range(HPC):
                        q_block(h)
                else:
                    # last panel: Q (long DVE rope chains) first so the final
                    # p1-psum consumers finish close to the last PE matmul
                    for h in range(HPC):
                        q_block(h)
                    k_block()
                    v_block()

        # -------- phase 2 + 3: attention + out projection (fused) --------
        # Single-key-block units: scores matmul -> exp -> (ctx matmul, paired
        # col-tiled rowsum matmuls). PSUM: sc 3x1 + cx 2x1 + rs 1x1 + p3 2x1.
        with ExitStack() as p2:
            e = p2.enter_context
            wo_pool = e(tc.tile_pool(name="wo", bufs=1))
            woT_sb = wo_pool.tile([128, HPC, D], BF, name="woT_sb")
            nc.sync.dma_start(woT_sb[:], woT[:])

            sc_psum = e(tc.tile_pool(name="scps", bufs=3, space="PSUM"))
            cx_psum = e(tc.tile_pool(name="cxps", bufs=2, space="PSUM"))
            rs_psum = e(tc.tile_pool(name="rsps", bufs=1, space="PSUM"))
            p3_psum = e(tc.tile_pool(name="p3ps", bufs=2, space="PSUM"))
            pt_pool = e(tc.tile_pool(name="pt", bufs=8))
            rs_pool = e(tc.tile_pool(name="rs", bufs=2))
            bc_pool = e(tc.tile_pool(name="bc", bufs=2))
            ev_pool = e(tc.tile_pool(name="ev", bufs=8))
            EXP = mybir.ActivationFunctionType.Exp

            def chunk_units(c, h):
                    q_rhs = qTc[c][:, h, :]
                    cps = cx_psum.tile([128, IC], F32, name="cxa", tag="cxa")
                    rps = rs_psum.tile([128, IC], F32, name="rsa", tag="rsa")
                    rs_started = [False]
                    rs_defer = []  # deferred (jb, pt, lo, stop) rowsums

                    def ensure_rs_start():
                        # Clear the rowsum bank with one full-width zero-
                        # stationary matmul: start=True clears has_written
                        # (bank-wide on HW, uniform pending-zero in CoreSim)
                        # and writes 0.0 everywhere; every rowsum matmul then
                        # accumulates with start=False. A DVE memset does NOT
                        # work here (silent no-op to PSUM on HW).
                        if not rs_started[0]:
                            nc.tensor.matmul(
                                rps[:, :], zeros_sb[:], kTc[c][:],
                                start=True, stop=False,
                                skip_group_check=True,
                            )
                        rs_started[0] = True

                    def flush_rs():
                        # adjacent rowsum matmuls in distinct column groups
                        # (partitions 0/32/64/96) -> 4-way col-tile overlap
                        for jb, pt, lo, stop in rs_defer:
                            g = 32 * (jb % RS_GROUPS)
                            nc.tensor.matmul(
                                rps[g : g + 1, lo:IC], ones_sb[:],
                                pt[:, lo:IC],
                                start=False, stop=stop,
                                skip_group_check=True,
                                tile_position=(0, g),
                            )
                        rs_defer.clear()

                    def scores_blk(jb):
                        diag = jb >= 4 * c
                        lo = (jb - 4 * c) * 128 if diag else 0
                        sps = sc_psum.tile([128, IC], F32, name="scp", tag="scp")
                        nc.tensor.matmul(
                            sps[:, lo:IC],
                            kTc[jb // 4][:, (jb % 4) * 128 : (jb % 4 + 1) * 128],
                            q_rhs[:, lo:IC],
                            start=True,
                            stop=True,
                        )
                        pt = pt_pool.tile([128, IC], BF, name="pt", tag="pt")
                        nc.scalar.activation(
                            pt[:, lo:IC], sps[:, lo:IC], EXP, scale=float(SCALE)
                        )
                        if diag:
                            nc.vector.tensor_mul(
                                pt[:, lo : lo + 128],
                                pt[:, lo : lo + 128],
                                mask_sb[:],
                            )
                        return pt

                    def tail_blk(jb, pt, is_last=False):
                        ensure_rs_start()
                        diag = jb >= 4 * c
                        lo = (jb - 4 * c) * 128 if diag else 0
                        first = jb == 0
                        nc.tensor.matmul(
                            cps[:, lo:IC], vnatc[jb // 4][:, jb % 4, :],
                            pt[:, lo:IC],
                            start=first, stop=is_last,
                            skip_group_check=True,
                        )
                        rs_defer.append((jb, pt, lo, is_last))
                        if len(rs_defer) == RS_GROUPS or is_last:
                            flush_rs()

                    def finalize():
                        rsf = rs_pool.tile([1, IC], F32, name="rsf", tag="rsf")
                        t01 = rs_pool.tile([1, IC], F32, name="t01", tag="t01")
                        # walrus: a TensorTensor may read only one PSUM
                        # input, so accumulate the group partials via SBUF
                        nc.vector.tensor_copy(t01[:], rps[0:1, :])
                        for gi in range(1, RS_GROUPS):
                            dst = rsf if gi == RS_GROUPS - 1 else t01
                            nc.vector.tensor_add(
                                dst[:], t01[:], rps[32 * gi : 32 * gi + 1, :]
                            )
                        nc.vector.reciprocal(rsf[:], rsf[:])
                        bc = bc_pool.tile([128, IC], F32, name="bc", tag="bc")
                        nc.gpsimd.partition_broadcast(bc[:], rsf[:])
                        nc.vector.tensor_mul(ctxc[c][:, h, :], cps[:], bc[:])

                    nblk = 4 * c + 4
                    units = [(scores_blk, tail_blk, jb) for jb in range(nblk)]
                    return units, finalize

            # phase-3 unit: one output block (128 rows x IC cols) of the
            # out-projection for chunk c — pure exp-independent PE work
            def p3_unit(c, ob):
                ps = p3_psum.tile([128, IC], F32, name="p3a", tag="p3a")
                for h in range(HPC):
                    nc.tensor.matmul(
                        ps[:],
                        woT_sb[:, h, ob * 128 : (ob + 1) * 128],
                        ctxc[c][:, h, :],
                        start=(h == 0),
                        stop=(h == HPC - 1),
                    )
                ev = ev_pool.tile([128, IC], BF, name="ev", tag="ev")
                if ob % 2 == 0:
                    nc.scalar.activation(
                        ev[:], ps[:], mybir.ActivationFunctionType.Copy
                    )
                else:
                    nc.vector.tensor_copy(ev[:], ps[:])
                nc.sync.dma_start(
                    outT[ob * 128 : (ob + 1) * 128, c * IC : c * IC + IC], ev[:]
                )

            # Global software pipeline, lookahead 3 across all chunks, with
            # phase-3 units of completed chunks interleaved as PE filler.
            from collections import deque

            pend = deque()
            p3_ready = deque()  # (c, next_ob)
            p3_credit = 0.0
            p3_backlog = [0]
            p3_reserve = [10]  # keep units in reserve for the last finalize

            def drain_p3(k, ignore_reserve=False):
                n = 0
                while (
                    n < k
                    and p3_ready
                    and (ignore_reserve or p3_backlog[0] > p3_reserve[0])
                ):
                    c0, ob = p3_ready[0]
                    p3_unit(c0, ob)
                    n += 1
                    p3_backlog[0] -= 1
                    if ob + 1 < NOB:
                        p3_ready[0] = (c0, ob + 1)
                    else:
                        p3_ready.popleft()
                return n

            n_units_total = sum(4 * c + 4 for c in range(NIC)) * HPC
            # slightly under-provision so a reserve of p3 units remains to
            # cover the final finalize window and the tail
            P3_PER_UNIT = (NIC * NOB) / float(n_units_total) if FUSE_P3 else 0.0

            done_heads = [0] * NIC

            def pop_pend():
                p = pend.popleft()
                p[1](p[2], p[0], is_last=p[3] is not None)
                if p[3] is not None:
                    p[3]()
                    cc = p[4]
                    done_heads[cc] += 1
                    if done_heads[cc] == HPC and FUSE_P3:
                        p3_ready.append((cc, 0))
                        p3_backlog[0] += NOB
                        if cc == NIC - 1:
                            p3_reserve[0] = 0

            for c in range(NIC):
                for h in range(HPC):
                    units, fin = chunk_units(c, h)
                    for i, (sc_fn, tl_fn, arg) in enumerate(units):
                        pt = sc_fn(arg)
                        if len(pend) >= 3:
                            pop_pend()
                        pend.append((pt, tl_fn, arg,
                                     fin if i == len(units) - 1 else None, c))
                        p3_credit += P3_PER_UNIT
                        if p3_credit >= 1.0:
                            # cap at 2 per step so a backlog burst never
                            # starves ACT of fresh score matmuls
                            p3_credit -= drain_p3(min(int(p3_credit), 3))
            while pend:
                pop_pend()
                drain_p3(2, ignore_reserve=p3_reserve[0] == 0)
            while p3_ready:
                drain_p3(4, ignore_reserve=True)

            if not FUSE_P3:
                for c in range(NIC):
                    for ob in range(NOB):
                        p3_unit(c, ob)


def _rope(nc, tc, rtmp_pool, ps, out_ap, cos_ap, sins_ap):
    """out = psum*cos + rot(psum)*sin_signed, written as bf16."""
    n = ps.shape[-1]
    tmp = rtmp_pool.tile([128, n], F32, name="ropetmp", tag="ropetmp")
    t2 = rtmp_pool.tile([128, n], F32, name="ropet2", tag="ropet2")
    nc.vector.tensor_mul(tmp[:], ps[:], cos_ap)
    nc.vector.tensor_mul(t2[:64, :], ps[64:, :], sins_ap[:64, :])
    nc.vector.tensor_mul(t2[64:, :], ps[:64, :], sins_ap[64:, :])
    nc.vector.tensor_add(out_ap, tmp[:], t2[:])


_NC_CACHE = None


def _get_nc():
    global _NC_CACHE
    if _NC_CACHE is None:
        _NC_CACHE = _build_nc()
    return _NC_CACHE


def prepare_in_maps(x, cos, sin, Wq, Wk, Wv, Wo):
    x = np.asarray(x, np.float32)
    cos = np.asarray(cos, np.float32)
    sin = np.asarray(sin, np.float32)
    Wq = np.asarray(Wq, np.float32)
    Wk = np.asarray(Wk, np.float32)
    Wv = np.asarray(Wv, np.float32)
    Wo = np.asarray(Wo, np.float32)

    import ml_dtypes

    bf = ml_dtypes.bfloat16
    xT_bf = np.ascontiguousarray(x[0].T).astype(bf)
    cosT = np.ascontiguousarray(cos.T).astype(bf)            # [128, S] bf16
    sinT = sin.T
    sinTs = np.ascontiguousarray(
        np.concatenate([-sinT[:64], sinT[64:]], axis=0)
    ).astype(bf)

    # causal triangle for a single [key 128, query 128] diagonal sub-block
    j = np.arange(128)[:, None]
    i = np.arange(128)[None, :]
    masks = (i >= j).astype(np.float32).astype(bf)

    def sbuf_layout(wT, width):
        return np.ascontiguousarray(
            wT.reshape(DCH, 128, width).transpose(1, 0, 2)
        ).astype(bf)

    in_maps = []
    for c in range(NCORES):
        wq_c = Wq[c * QO : (c + 1) * QO]          # [512, 4096]
        wk_c = Wk[c * HD : (c + 1) * HD]          # [128, 4096]
        wv_c = Wv[c * HD : (c + 1) * HD]
        wo_c = Wo[:, c * QO : (c + 1) * QO]       # [4096, 512]
        woT_pre = np.ascontiguousarray(
            wo_c.T.reshape(HPC, 128, D).transpose(1, 0, 2)
        ).astype(bf)                               # [128, HPC, D]
        in_maps.append(
            {
                "xT": xT_bf,
                "cosT": cosT,
                "sinTs": sinTs,
                "wqT": np.ascontiguousarray(
                    sbuf_layout(wq_c.T, QO)
                    .reshape(128, DCH, HPC, HD)
                    .transpose(2, 0, 1, 3)
                ),
                "wkT": sbuf_layout(wk_c.T, HD),
                "wvT": sbuf_layout(wv_c.T, HD),
                "woT": woT_pre,
                "masks": masks,
            }
        )
    return in_maps


def kernel(x, cos, sin, Wq, Wk, Wv, Wo, _trace=False):
    nc = _get_nc()
    in_maps = prepare_in_maps(x, cos, sin, Wq, Wk, Wv, Wo)
    res = run_bass_kernel_spmd(
        nc, in_maps, core_ids=list(range(NCORES)), trace=_trace
    )
    acc = np.zeros((D, S), np.float32)
    for r in res.results:
        acc += r["outT"].astype(np.float32)
    out = np.ascontiguousarray(acc.T)[None]      # [1, S, D] fp32
    if _trace:
        return out, res
    return out
